# revision 9
# baseline (speedup 1.0000x reference)
"""MLA-style attention (shared latent KV head, attention sink, partial RoPE,
low-rank Q and grouped low-rank output projection) on 8 TRN2 NeuronCores.

Sharding: 64 query heads split 8 per core (tensor parallel on wq_b rows /
wo_a groups); latent KV path seq-sharded then all-gathered; final wo_b
matmul computed as per-core partial products summed on the host.

v2 restructure vs baseline:
- collective gathers qrT+kv only (kvT derived locally by transposes)
- max-free softmax: logits bounded by sqrt(HD) (q,kv rms-normed), so no
  row-max pass; q-RMS scale folded into the exp's per-partition scale AP
- RoPE batched across all seq tiles (6 DVE ops instead of 48 per head)
- stage E streams wo_b in quarters and DMAs outputs straight from PSUM
"""

import numpy as np
import ml_dtypes

import concourse.bass as bass
import concourse.mybir as mybir
import concourse.tile as tile
from concourse import bacc
from concourse.bass_utils import run_bass_kernel_spmd
from concourse.masks import make_identity, make_causal_mask

BF16 = mybir.dt.bfloat16
F32 = mybir.dt.float32
AX = mybir.AxisListType
ALU = mybir.AluOpType
ACTF = mybir.ActivationFunctionType

NPBF16 = ml_dtypes.bfloat16

# problem dims (hardcoded; kernel.py must be self-contained)
D, NH, HD, RD, QLR, OLR, OG = 4096, 64, 512, 64, 1024, 1024, 8
S = 1024
NCORES = 8
HPC = NH // NCORES  # query heads per core
EPS = 1e-6
P = 128


class Cfg:
    def __init__(self, s=S, d=D, qlr=QLR, hpc=HPC, olr=OLR, outd=D):
        assert s % P == 0 and d % P == 0 and qlr % 512 == 0 and olr % 512 == 0
        assert outd % 512 == 0
        self.s, self.d, self.qlr, self.hpc, self.olr, self.outd = (
            s, d, qlr, hpc, olr, outd)
        self.sc = s // P        # seq tiles
        self.dc = d // P        # model-dim chunks (contraction for qr/kv)
        self.qc = qlr // P      # q_lora chunks
        self.hc = HD // P       # head-dim chunks (4)
        self.f = hpc * HD       # per-core attention output feature dim
        self.fc = self.f // P   # feature chunks for wo_a contraction
        self.oc = olr // P      # olr chunks (contraction for wo_b)
        self.nc_out = outd // 512  # output D chunks


def _rope_tail(nc, pool, dst, cos_ap, sin_ap, inverse, tag):
    """Partial RoPE on dst[..., HD-RD:HD] in place.

    dst: [128, HD] or [128, sc, HD] bf16; cos/sin: matching [128, RD//2] or
    [128, sc, RD//2] f32."""
    if len(dst.shape) == 3:
        tail = dst[:, :, HD - RD:HD].rearrange("p s (a two) -> p s a two",
                                               two=2)
        x1 = tail[:, :, :, 0]
        x2 = tail[:, :, :, 1]
        tshape = [P, dst.shape[1], RD // 2]
    else:
        tail = dst[:, HD - RD:HD].rearrange("p (a two) -> p a two", two=2)
        x1 = tail[:, :, 0]
        x2 = tail[:, :, 1]
        tshape = [P, RD // 2]
    nd = len(tshape)
    t1 = pool.tile(tshape, F32, tag=f"rope{nd}_1", bufs=1)
    t2 = pool.tile(tshape, F32, tag=f"rope{nd}_2", bufs=1)
    t3 = pool.tile(tshape, F32, tag=f"rope{nd}_3", bufs=1)
    t4 = pool.tile(tshape, F32, tag=f"rope{nd}_4", bufs=1)
    nc.vector.tensor_mul(t1[:], x1, cos_ap)
    nc.vector.tensor_mul(t2[:], x2, sin_ap)
    nc.vector.tensor_mul(t3[:], x1, sin_ap)
    nc.vector.tensor_mul(t4[:], x2, cos_ap)
    if not inverse:
        nc.vector.tensor_sub(x1, t1[:], t2[:])
        nc.vector.tensor_add(x2, t3[:], t4[:])
    else:
        nc.vector.tensor_add(x1, t1[:], t2[:])
        nc.vector.tensor_sub(x2, t4[:], t3[:])


def build_program(cfg: Cfg, debug=False, reps=1):
    nc = bacc.Bacc("TRN2", debug=False, num_devices=NCORES)

    # ---- DRAM I/O (host supplies pre-tiled layouts) ----
    xt_d = nc.dram_tensor("xtm", [P, cfg.dc, P], BF16,
                          kind="ExternalInput").ap()
    cosm_d = nc.dram_tensor("cosm", [P, RD // 2], F32,
                            kind="ExternalInput").ap()
    sinm_d = nc.dram_tensor("sinm", [P, RD // 2], F32,
                            kind="ExternalInput").ap()
    wqa_d = nc.dram_tensor("wqa", [P, cfg.dc, cfg.qlr], BF16,
                           kind="ExternalInput").ap()
    wkv_d = nc.dram_tensor("wkv", [P, cfg.dc, HD], BF16,
                           kind="ExternalInput").ap()
    wqb_d = nc.dram_tensor("wqb", [P, cfg.qc, cfg.hpc * HD], BF16,
                           kind="ExternalInput").ap()
    woa_d = nc.dram_tensor("woa", [P, cfg.fc, cfg.olr], BF16,
                           kind="ExternalInput").ap()
    wob_d = nc.dram_tensor("wob", [P, cfg.oc, cfg.outd], BF16,
                           kind="ExternalInput").ap()
    cos_d = nc.dram_tensor("coss", [P, cfg.sc, RD // 2], F32,
                           kind="ExternalInput").ap()
    sin_d = nc.dram_tensor("sins", [P, cfg.sc, RD // 2], F32,
                           kind="ExternalInput").ap()
    kvw_d = nc.dram_tensor("kvw", [P, HD], BF16, kind="ExternalInput").ap()
    sinkexp_d = nc.dram_tensor("sinkexp", [P, cfg.hpc], F32,
                               kind="ExternalInput").ap()
    out_d = nc.dram_tensor("out", [cfg.sc, P, cfg.outd], F32,
                           kind="ExternalOutput").ap()

    with tile.TileContext(nc) as tc:
        for _ in range(reps):
            _body(nc, tc, cfg, xt_d, wqa_d, wkv_d, wqb_d, woa_d, wob_d,
                  cos_d, sin_d, kvw_d, sinkexp_d, out_d, cosm_d, sinm_d)

    nc.compile()
    return nc


def _body(nc, tc, cfg, xt_d, wqa_d, wkv_d, wqb_d, woa_d, wob_d,
          cos_d, sin_d, kvw_d, sinkexp_d, out_d, cosm_d, sinm_d):
    sc, dc, qc, hc = cfg.sc, cfg.dc, cfg.qc, cfg.hc

    with tc.tile_pool(name="persist", bufs=1) as pp:
        ident = pp.tile([P, P], BF16)
        make_identity(nc, ident[:])
        cmask = pp.tile([P, P], F32)
        make_causal_mask(nc, cmask[:], mask_val=-1e10)
        kvw_sb = pp.tile([P, HD], BF16)
        nc.gpsimd.dma_start(kvw_sb[:], kvw_d)
        sinkexp_sb = pp.tile([P, cfg.hpc], F32)
        nc.gpsimd.dma_start(sinkexp_sb[:], sinkexp_d)
        cos_sb = pp.tile([P, sc, RD // 2], F32)
        nc.gpsimd.dma_start(cos_sb[:], cos_d)
        sin_sb = pp.tile([P, sc, RD // 2], F32)
        nc.gpsimd.dma_start(sin_sb[:], sin_d)
        cosm_sb = pp.tile([P, RD // 2], F32)
        nc.gpsimd.dma_start(cosm_sb[:], cosm_d)
        sinm_sb = pp.tile([P, RD // 2], F32)
        nc.gpsimd.dma_start(sinm_sb[:], sinm_d)
        kv_sb = pp.tile([P, sc, HD], BF16)      # latent KV, [s-in-tile, tile, hd]
        kvT_sb = pp.tile([P, hc, cfg.s], BF16)  # latent KV transposed
        eps_sb = pp.tile([P, 2], F32)           # [:,0]=EPS, [:,1]=-ln(HD)/2
        nc.gpsimd.memset(eps_sb[:, 0:1], float(EPS))
        nc.gpsimd.memset(eps_sb[:, 1:2], float(-0.5 * np.log(HD)))

        with tc.tile_pool(name="qrt", bufs=1) as qrtp:
            qrT_sb = qrtp.tile([P, qc, cfg.s], BF16)

            # ================= stage A: local qr + kv slice ================
            with tc.tile_pool(name="stA", bufs=1) as pa, \
                 tc.tile_pool(name="stAw", bufs=2) as paw, \
                 tc.tile_pool(name="psA", bufs=1, space="PSUM") as psa:
                xt_i = paw.tile([P, dc, P], BF16, tag="xt")
                nc.sync.dma_start(xt_i[:], xt_d)
                wqa_sb = pa.tile([P, dc, cfg.qlr], BF16)
                nsp = min(8, dc)
                for g in range(nsp):
                    gsz = dc // nsp
                    nc.gpsimd.dma_start(wqa_sb[:, g * gsz:(g + 1) * gsz, :],
                                        wqa_d[:, g * gsz:(g + 1) * gsz, :])
                wkv_sb = pa.tile([P, dc, HD], BF16)
                nsp = min(4, dc)
                for g in range(nsp):
                    gsz = dc // nsp
                    nc.gpsimd.dma_start(wkv_sb[:, g * gsz:(g + 1) * gsz, :],
                                        wkv_d[:, g * gsz:(g + 1) * gsz, :])
                qr_ps = psa.tile([P, cfg.qlr], F32, tag="qr", bufs=1)
                kv_ps = psa.tile([P, HD], F32, tag="kv", bufs=1)
                for k in range(dc):
                    st, sp = k == 0, k == dc - 1
                    for n2 in range(cfg.qlr // 512):
                        nc.tensor.matmul(
                            qr_ps[:, n2 * 512:(n2 + 1) * 512],
                            xt_i[:, k, :],
                            wqa_sb[:, k, n2 * 512:(n2 + 1) * 512],
                            start=st, stop=sp)
                    nc.tensor.matmul(kv_ps[:], xt_i[:, k, :],
                                     wkv_sb[:, k, :], start=st, stop=sp)

                # --- qr epilogue: cast, rmsnorm, transpose ---
                qr_sb = paw.tile([P, cfg.qlr], BF16, tag="qr_sb")
                nc.any.tensor_copy(qr_sb[:], qr_ps[:])
                sq = paw.tile([P, cfg.qlr], F32, tag="sq")
                ssq = paw.tile([P, 1], F32, tag="ssq")
                nc.scalar.activation(sq[:], qr_sb[:], ACTF.Square,
                                     accum_out=ssq[:])
                rt = paw.tile([P, 1], F32, tag="rt")
                nc.scalar.activation(rt[:], ssq[:], ACTF.Ln,
                                     bias=eps_sb[:, 0:1],
                                     scale=1.0 / cfg.qlr)
                rinv = paw.tile([P, 1], F32, tag="rinv")
                nc.scalar.activation(rinv[:], rt[:], ACTF.Exp, scale=-0.5)
                qrn = paw.tile([P, cfg.qlr], BF16, tag="qrn")
                nc.scalar.mul(qrn[:], qr_sb[:], rinv[:])
                qrT_loc = paw.tile([P, qc, P], BF16, tag="qrT_loc", bufs=1)
                nc.sync.dma_start_transpose(qrT_loc[:], qrn[:])

                # --- kv epilogue: cast, rmsnorm, weight, rope ---
                kvt = paw.tile([P, HD], BF16, tag="kvt")
                nc.any.tensor_copy(kvt[:], kv_ps[:])
                sqk = paw.tile([P, HD], F32, tag="sqk")
                ssqk = paw.tile([P, 1], F32, tag="ssqk")
                nc.scalar.activation(sqk[:], kvt[:], ACTF.Square,
                                     accum_out=ssqk[:])
                rtk = paw.tile([P, 1], F32, tag="rtk")
                nc.scalar.activation(rtk[:], ssqk[:], ACTF.Ln,
                                     bias=eps_sb[:, 0:1], scale=1.0 / HD)
                rinvk = paw.tile([P, 1], F32, tag="rinvk")
                nc.scalar.activation(rinvk[:], rtk[:], ACTF.Exp, scale=-0.5)
                kv_loc = paw.tile([P, HD], BF16, tag="kv_loc", bufs=1)
                nc.scalar.mul(kv_loc[:], kvt[:], rinvk[:])
                nc.vector.tensor_mul(kv_loc[:], kv_loc[:], kvw_sb[:])
                _rope_tail(nc, paw, kv_loc[:], cosm_sb[:], sinm_sb[:],
                           False, tag="rkv")

                # pack local results into DRAM and all-gather (qrT + kv)
                gw = qc * P + HD      # 1536
                with tc.tile_pool(name="ccdram", bufs=1, space="DRAM") as ccd:
                    gin = ccd.tile([P, gw], BF16)
                    gout = ccd.tile([NCORES, P, gw], BF16,
                                    addr_space="Shared")
                    nc.sync.dma_start(
                        gin[:, 0:qc * P],
                        qrT_loc[:].rearrange("p c s -> p (c s)"))
                    nc.sync.dma_start(gin[:, qc * P:gw], kv_loc[:])
                    nc.gpsimd.collective_compute(
                        "AllGather", ALU.bypass,
                        replica_groups=[list(range(NCORES))],
                        ins=[gin[:]], outs=[gout[:]])
                    for j in range(NCORES):
                        nc.sync.dma_start(
                            qrT_sb[:, :, j * P:(j + 1) * P],
                            gout[j, :, 0:qc * P].rearrange(
                                "p (c s) -> p c s", c=qc))
                        nc.sync.dma_start(kv_sb[:, j, :],
                                          gout[j, :, qc * P:gw])

                # kvT derived locally from gathered kv (xbar transpose)
                for j in range(sc):
                    nc.sync.dma_start_transpose(
                        kvT_sb[:, :, j * P:(j + 1) * P].rearrange(
                            "p c s -> p c s"),
                        kv_sb[:, j, :])

            # ====== stage BC: per-head q proj + attention + wo_a partial ====
            s_chunks = [(a, min(512, cfg.s - a))
                        for a in range(0, cfg.s, 512)]
            with tc.tile_pool(name="og", bufs=1) as ogp, \
                 tc.tile_pool(name="stEw", bufs=2) as pew:
                og_acc = ogp.tile([P, cfg.oc, cfg.s], F32)
                ogT_sb = ogp.tile([P, cfg.oc, cfg.s], BF16)
                # wo_b eighths: no input deps, first loads during stage BC
                wob_qs = []
                for quarter in range(cfg.nc_out):
                    wob_q = pew.tile([P, cfg.oc, 512], BF16, tag="wobq")
                    nc.gpsimd.dma_start(
                        wob_q[:],
                        wob_d[:, :, quarter * 512:(quarter + 1) * 512])
                    wob_qs.append(wob_q)

                with tc.tile_pool(name="stBC", bufs=1) as pb, \
                     tc.tile_pool(name="stBCw", bufs=2) as pbw, \
                     tc.tile_pool(name="psQ", bufs=1, space="PSUM") as psq, \
                     tc.tile_pool(name="psS", bufs=1, space="PSUM") as pss, \
                     tc.tile_pool(name="psT", bufs=1, space="PSUM") as pst, \
                     tc.tile_pool(name="psO", bufs=1, space="PSUM") as pso, \
                     tc.tile_pool(name="psD", bufs=1, space="PSUM") as psd:
                    for h in range(cfg.hpc):
                        woa_h = pbw.tile([P, hc, cfg.olr], BF16, tag="woa_h")
                        nc.gpsimd.dma_start(
                            woa_h[:], woa_d[:, h * hc:(h + 1) * hc, :])
                        wqb_h = pbw.tile([P, qc, HD], BF16, tag="wqb_h")
                        nc.gpsimd.dma_start(
                            wqb_h[:], wqb_d[:, :, h * HD:(h + 1) * HD])
                        qT_sb = pbw.tile([P, hc, cfg.s], BF16, tag="qT")
                        # ---- q projection (q left unnormalized; the RMS
                        # scale is folded into the softmax exp below) ----
                        q8 = pbw.tile([P, sc, HD], BF16, tag="q8", bufs=1)
                        ssq8 = pbw.tile([P, sc], F32, tag="ssq8")
                        for i in range(sc):
                            q_ps = psq.tile([P, HD], F32, tag="q", bufs=2)
                            for c in range(qc):
                                nc.tensor.matmul(
                                    q_ps[:],
                                    qrT_sb[:, c, i * P:(i + 1) * P],
                                    wqb_h[:, c, :],
                                    start=(c == 0), stop=(c == qc - 1))
                            nc.any.tensor_copy(q8[:, i, :], q_ps[:])
                            sqq = pbw.tile([P, HD], F32, tag="sqq", bufs=1)
                            nc.scalar.activation(sqq[:], q8[:, i, :],
                                                 ACTF.Square,
                                                 accum_out=ssq8[:, i:i + 1])
                        # rsqrt(ms+eps)/sqrt(HD) = exp(-.5 ln(ssq/HD+eps)
                        #                              -.5 ln(HD))
                        rt8 = pbw.tile([P, sc], F32, tag="rt8")
                        nc.scalar.activation(rt8[:], ssq8[:], ACTF.Ln,
                                             bias=eps_sb[:, 0:1],
                                             scale=1.0 / HD)
                        rinv8 = pbw.tile([P, sc], F32, tag="rinv8")
                        nc.scalar.activation(rinv8[:], rt8[:], ACTF.Exp,
                                             scale=-0.5,
                                             bias=eps_sb[:, 1:2])
                        _rope_tail(nc, pbw, q8[:], cos_sb[:], sin_sb[:],
                                   False, tag="rq")
                        for i in range(sc):
                            tpq = pst.tile([P, 512], BF16, tag="t", bufs=1)
                            for c in range(hc):
                                nc.tensor.transpose(
                                    tpq[:, c * P:(c + 1) * P],
                                    q8[:, i, c * P:(c + 1) * P], ident[:])
                            nc.any.tensor_copy(
                                qT_sb[:, :, i * P:(i + 1) * P],
                                tpq[:].rearrange("p (c s) -> p c s", c=hc))

                        # ---- attention for head h (max-free softmax) ----
                        o8 = pbw.tile([P, sc, HD], BF16, tag="o8", bufs=1)
                        for i in range(sc):
                            w_all = (i + 1) * P
                            nch = (w_all + 511) // 512
                            s_ps = []
                            for ci in range(nch):
                                wci = min(512, w_all - ci * 512)
                                s_ps.append((pss.tile([P, 512], F32, tag="s",
                                                      bufs=2, name="s_ps"),
                                             wci))
                            for k in range(hc):
                                for ci in range(nch):
                                    tile_ps, wci = s_ps[ci]
                                    nc.tensor.matmul(
                                        tile_ps[:, :wci],
                                        qT_sb[:, k, i * P:(i + 1) * P],
                                        kvT_sb[:, k, ci * 512:ci * 512 + wci],
                                        start=(k == 0), stop=(k == hc - 1))
                            # causal mask on the diagonal block
                            dps, dw = s_ps[-1]
                            dcol = (w_all - P) - (nch - 1) * 512
                            nc.vector.tensor_add(dps[:, dcol:dcol + P],
                                                 dps[:, dcol:dcol + P],
                                                 cmask[:])
                            # exp with the q-RMS scale folded in; row sums
                            # accumulate per chunk (no max subtraction:
                            # |logit| <= sqrt(HD) so exp stays in f32 range)
                            p_sb = pbw.tile([P, cfg.s], BF16, tag="p")
                            l0 = pbw.tile([P, 2], F32, tag="l0")
                            for ci in range(nch):
                                tile_ps, wci = s_ps[ci]
                                nc.scalar.activation(
                                    p_sb[:, ci * 512:ci * 512 + wci],
                                    tile_ps[:, :wci], ACTF.Exp,
                                    scale=rinv8[:, i:i + 1],
                                    accum_out=l0[:, ci:ci + 1])
                            lsum = pbw.tile([P, 1], F32, tag="lsum")
                            if nch == 1:
                                nc.vector.tensor_add(
                                    lsum[:], l0[:, 0:1],
                                    sinkexp_sb[:, h:h + 1])
                            else:
                                nc.vector.tensor_add(lsum[:], l0[:, 0:1],
                                                     l0[:, 1:2])
                                nc.vector.tensor_add(
                                    lsum[:], lsum[:],
                                    sinkexp_sb[:, h:h + 1])
                            linv = pbw.tile([P, 1], F32, tag="linv")
                            nc.vector.reciprocal(linv[:], lsum[:])
                            # transpose p
                            pT_sb = pbw.tile([P, cfg.s], BF16, tag="pT")
                            for g in range((i + 1 + 3) // 4):
                                jn = min(4, (i + 1) - g * 4)
                                tpp = pst.tile([P, 512], BF16, tag="t",
                                               bufs=1)
                                for j4 in range(jn):
                                    j = g * 4 + j4
                                    nc.tensor.transpose(
                                        tpp[:, j4 * P:(j4 + 1) * P],
                                        p_sb[:, j * P:(j + 1) * P], ident[:])
                                nc.any.tensor_copy(
                                    pT_sb[:, g * 512:g * 512 + jn * P],
                                    tpp[:, :jn * P])
                            # o = p^T-weighted sum of kv rows
                            o_ps = pso.tile([P, HD], F32, tag="o", bufs=1)
                            for j in range(i + 1):
                                nc.tensor.matmul(o_ps[:],
                                                 pT_sb[:, j * P:(j + 1) * P],
                                                 kv_sb[:, j, :],
                                                 start=(j == 0),
                                                 stop=(j == i))
                            # normalize into o8 (batched inv-rope later)
                            nc.vector.tensor_scalar_mul(o8[:, i, :], o_ps[:],
                                                        linv[:])

                        _rope_tail(nc, pbw, o8[:], cos_sb[:], sin_sb[:],
                                   True, tag="ro")
                        oT_h = pbw.tile([P, hc, cfg.s], BF16, tag="oT_h")
                        for i in range(sc):
                            tpo = pst.tile([P, 512], BF16, tag="t", bufs=1)
                            for c in range(hc):
                                nc.tensor.transpose(
                                    tpo[:, c * P:(c + 1) * P],
                                    o8[:, i, c * P:(c + 1) * P], ident[:])
                            nc.any.tensor_copy(
                                oT_h[:, :, i * P:(i + 1) * P],
                                tpo[:].rearrange("p (c s) -> p c s", c=hc))

                        # ---- wo_a partial for this head, into f32 og_acc --
                        for m in range(cfg.oc):
                            d_ps = []
                            for n2 in range(len(s_chunks)):
                                d_ps.append(psd.tile([P, 512], F32,
                                                     tag=f"d{n2}", bufs=1,
                                                     name="d_ps"))
                            for kk in range(hc):
                                for n2, (a, w) in enumerate(s_chunks):
                                    nc.tensor.matmul(
                                        d_ps[n2][:, :w],
                                        woa_h[:, kk, m * P:(m + 1) * P],
                                        oT_h[:, kk, a:a + w],
                                        start=(kk == 0), stop=(kk == hc - 1))
                            for n2, (a, w) in enumerate(s_chunks):
                                if h == 0:
                                    nc.vector.tensor_copy(
                                        og_acc[:, m, a:a + w],
                                        d_ps[n2][:, :w])
                                elif h == cfg.hpc - 1:
                                    # final add writes bf16 ogT directly
                                    nc.vector.tensor_add(
                                        ogT_sb[:, m, a:a + w],
                                        og_acc[:, m, a:a + w],
                                        d_ps[n2][:, :w])
                                else:
                                    nc.vector.tensor_add(
                                        og_acc[:, m, a:a + w],
                                        og_acc[:, m, a:a + w],
                                        d_ps[n2][:, :w])

                # ============ stage E: final wo_b partial matmul ===========
                with tc.tile_pool(name="psE", bufs=1, space="PSUM") as pse:
                    for quarter in range(cfg.nc_out):
                        wob_q = wob_qs[quarter]
                        for m in range(sc):
                            out_ps = pse.tile([P, 512], F32, tag="out",
                                              bufs=8, name="out_ps")
                            for k in range(cfg.oc):
                                nc.tensor.matmul(
                                    out_ps[:],
                                    ogT_sb[:, k, m * P:(m + 1) * P],
                                    wob_q[:, k, :],
                                    start=(k == 0), stop=(k == cfg.oc - 1))
                            o_out = pew.tile([P, 512], F32, tag="oo",
                                             bufs=4)
                            nc.any.tensor_copy(o_out[:], out_ps[:])
                            nc.sync.dma_start(
                                out_d[m, :,
                                      quarter * 512:(quarter + 1) * 512],
                                o_out[:])


# ---------------------------------------------------------------------------
# host side
# ---------------------------------------------------------------------------

def _pack_kt(w, n_rows, n_cols):
    """Pack W (given as [n_cols, n_rows] np array) into [128, n_rows/128,
    n_cols] = W.T tiled with the contraction dim on partitions."""
    wt = np.ascontiguousarray(w.T)  # [n_rows, n_cols]
    return np.ascontiguousarray(
        wt.reshape(n_rows // P, P, n_cols).transpose(1, 0, 2))


def prepare_inmaps(inputs, cfg: Cfg):
    bf = NPBF16
    x = np.asarray(inputs["x"], dtype=bf).reshape(cfg.s, cfg.d)
    xt = np.ascontiguousarray(
        x.T.reshape(cfg.dc, P, cfg.sc, P).transpose(2, 1, 0, 3))

    wq_a = np.asarray(inputs["wq_a"], dtype=bf)
    wqa = _pack_kt(wq_a, cfg.d, cfg.qlr)

    wkv = _pack_kt(np.asarray(inputs["wkv"], dtype=bf), cfg.d, HD)

    q_norm_w = np.asarray(inputs["q_norm_w"], dtype=np.float32)
    wq_b = np.asarray(inputs["wq_b"], dtype=bf).astype(np.float32)
    wq_b = (wq_b * q_norm_w[None, :]).astype(bf)  # fold q_norm into wq_b

    kv_norm_w = np.asarray(inputs["kv_norm_w"], dtype=bf)
    kvw = np.ascontiguousarray(np.broadcast_to(kv_norm_w, (P, HD)))

    cos = np.asarray(inputs["cos"], dtype=np.float32)
    sin = np.asarray(inputs["sin"], dtype=np.float32)
    cos_p = np.ascontiguousarray(
        cos.reshape(cfg.sc, P, RD // 2).transpose(1, 0, 2))
    sin_p = np.ascontiguousarray(
        sin.reshape(cfg.sc, P, RD // 2).transpose(1, 0, 2))

    wo_a = np.asarray(inputs["wo_a"], dtype=bf)  # [OG*OLR, F]
    wo_b = np.asarray(inputs["wo_b"], dtype=bf)  # [D, OG*OLR]
    sink = np.asarray(inputs["attn_sink"], dtype=np.float32)

    in_maps = []
    for c in range(NCORES):
        h0 = c * cfg.hpc
        wqb_c = wq_b[h0 * HD:(h0 + cfg.hpc) * HD, :]  # [hpc*HD, qlr]
        woa_c = wo_a[c * cfg.olr:(c + 1) * cfg.olr, :]  # [olr, F]
        wob_c = wo_b[:, c * cfg.olr:(c + 1) * cfg.olr]  # [outd, olr]
        sinkexp_c = np.exp(sink[h0:h0 + cfg.hpc])
        in_maps.append({
            "xtm": np.ascontiguousarray(xt[c]),
            "cosm": np.ascontiguousarray(cos_p[:, c, :]),
            "sinm": np.ascontiguousarray(sin_p[:, c, :]),
            "wqa": wqa,
            "wkv": wkv,
            "wqb": _pack_kt(wqb_c, cfg.qlr, cfg.hpc * HD),
            "woa": _pack_kt(woa_c, cfg.f, cfg.olr),
            "wob": _pack_kt(wob_c, cfg.olr, cfg.outd),
            "coss": cos_p,
            "sins": sin_p,
            "kvw": kvw,
            "sinkexp": np.ascontiguousarray(
                np.broadcast_to(sinkexp_c, (P, cfg.hpc))).astype(np.float32),
        })
    return in_maps


_CACHE = {}


def _get_program():
    if "nc" not in _CACHE:
        _CACHE["nc"] = build_program(Cfg())
    return _CACHE["nc"]


def run(inputs, trace=False):
    """Returns (output [1,S,D] bf16, BassKernelResults)."""
    cfg = Cfg()
    nc = _get_program()
    in_maps = prepare_inmaps(inputs, cfg)
    res = run_bass_kernel_spmd(nc, in_maps, core_ids=list(range(NCORES)),
                               trace=trace)
    acc = np.zeros((cfg.s, cfg.outd), np.float32)
    for r in res.results:
        acc += r["out"].reshape(cfg.s, cfg.outd)
    out = acc.astype(NPBF16).reshape(1, cfg.s, cfg.outd)
    return out, res


def kernel(**inputs) -> np.ndarray:
    out, _ = run(inputs)
    return out
